# revision 1
# baseline (speedup 1.0000x reference)
"""Trainium2 Bass kernel for nn_EuclideanDistanceHashDecoder.

Computes, for each edge (u, v):
    sigmoid(1 - || z_u/||z_u|| - z_v/||z_v|| + eps ||)
  = sigmoid(1 - sqrt(2 - 2*cos(z_u, z_v)) )   (eps terms ~1e-6, negligible)

Strategy (8 NeuronCores, data-parallel over edges):
  - z is cast to bf16 on the host (storage format choice; output error ~1e-4,
    far inside tolerance) and replicated to every core's DRAM.
  - Each core processes 150000/8 = 18750 edges as 147 tiles of 128 edges.
  - Per tile: two indirect row gathers (z[src], z[dst]) -> [128, 512] bf16,
    then three fused multiply-reduce ops (scalar_tensor_tensor with accum)
    produce sum(a*a), sum(b*b), sum(a*b) per edge. The squares are split
    between the Vector and Scalar engines to balance load.
  - One vectorized epilogue over [128, 147] computes
    sigmoid(1 - sqrt(2)*sqrt(1 - clamp(dot*rsqrt(qa*qb), <=1))).
  - Host reassembles per-core [128, T] outputs back into edge order.
"""
import numpy as np
import ml_dtypes

import concourse.bass as bass
import concourse.bacc as bacc
import concourse.mybir as mybir
import concourse.tile as tile
from concourse.bass_utils import run_bass_kernel_spmd

P = 128
DIM = 512
N_NODES = 50000
N_EDGES = 150000
N_CORES = 8
EPC = N_EDGES // N_CORES          # 18750 edges per core
T = (EPC + P - 1) // P            # 147 tiles per core
KB = 21                           # tiles per gather batch
NB = T // KB                      # 7 batches
F32 = mybir.dt.float32
BF16 = mybir.dt.bfloat16
SQRT2 = 1.4142135623730951

_cache = {}


def _build():
    """Build + compile the SPMD Bass program (one program, 8 cores)."""
    nc = bacc.Bacc("TRN2", target_bir_lowering=False, debug=True)
    z = nc.declare_dram_parameter("z", [N_NODES, DIM], BF16, isOutput=False)
    ia = nc.declare_dram_parameter("ia", [P, T], mybir.dt.int32, isOutput=False)
    ib = nc.declare_dram_parameter("ib", [P, T], mybir.dt.int32, isOutput=False)
    out = nc.declare_dram_parameter("out", [P, T], F32, isOutput=True)

    with tile.TileContext(nc) as tc:
        with (
            tc.tile_pool(name="idx", bufs=1) as idxp,
            tc.tile_pool(name="rows", bufs=6) as rowp,
            tc.tile_pool(name="acc", bufs=1) as accp,
        ):
            ia_s = idxp.tile([P, T], mybir.dt.int32)
            ib_s = idxp.tile([P, T], mybir.dt.int32)
            nc.sync.dma_start(out=ia_s[:], in_=ia[:])
            nc.sync.dma_start(out=ib_s[:], in_=ib[:])

            qa = accp.tile([P, T], F32, tag="qa")
            qb = accp.tile([P, T], F32, tag="qb")
            dd = accp.tile([P, T], F32, tag="dd")

            for t in range(T):
                at = rowp.tile([P, DIM], BF16, tag="a")
                bt = rowp.tile([P, DIM], BF16, tag="b")
                nc.gpsimd.indirect_dma_start(
                    out=at[:], out_offset=None, in_=z[:],
                    in_offset=bass.IndirectOffsetOnAxis(
                        ap=ia_s[:, t : t + 1], axis=0))
                nc.gpsimd.indirect_dma_start(
                    out=bt[:], out_offset=None, in_=z[:],
                    in_offset=bass.IndirectOffsetOnAxis(
                        ap=ib_s[:, t : t + 1], axis=0))

                junk = rowp.tile([P, DIM], BF16, tag="junk")
                sqf = rowp.tile([P, DIM], F32, tag="sqf")
                # dot always on DVE
                nc.vector.scalar_tensor_tensor(
                    out=junk[:], in0=at[:], scalar=1.0, in1=bt[:],
                    op0=mybir.AluOpType.mult, op1=mybir.AluOpType.mult,
                    accum_out=dd[:, t : t + 1])
                # squares: alternate DVE/ACT to balance engines
                if t % 2 == 0:
                    nc.scalar.activation(
                        out=sqf[:], in_=at[:],
                        func=mybir.ActivationFunctionType.Square,
                        accum_out=qa[:, t : t + 1])
                    nc.scalar.activation(
                        out=sqf[:], in_=bt[:],
                        func=mybir.ActivationFunctionType.Square,
                        accum_out=qb[:, t : t + 1])
                else:
                    nc.vector.scalar_tensor_tensor(
                        out=junk[:], in0=at[:], scalar=1.0, in1=at[:],
                        op0=mybir.AluOpType.mult, op1=mybir.AluOpType.mult,
                        accum_out=qa[:, t : t + 1])
                    nc.scalar.activation(
                        out=sqf[:], in_=bt[:],
                        func=mybir.ActivationFunctionType.Square,
                        accum_out=qb[:, t : t + 1])

            # epilogue over [128, T]
            p = accp.tile([P, T], F32, tag="p")
            nc.vector.tensor_mul(out=p[:], in0=qa[:], in1=qb[:])
            s = accp.tile([P, T], F32, tag="s")
            nc.scalar.activation(out=s[:], in_=p[:],
                                 func=mybir.ActivationFunctionType.Sqrt)
            r = accp.tile([P, T], F32, tag="r")
            nc.vector.reciprocal(out=r[:], in_=s[:])
            cos = accp.tile([P, T], F32, tag="cos")
            nc.vector.tensor_mul(out=cos[:], in0=dd[:], in1=r[:])
            nc.vector.tensor_scalar_min(out=cos[:], in0=cos[:], scalar1=1.0)
            u = accp.tile([P, T], F32, tag="u")
            nc.scalar.activation(out=u[:], in_=cos[:],
                                 func=mybir.ActivationFunctionType.Sqrt,
                                 scale=-1.0, bias=1.0)
            res = accp.tile([P, T], F32, tag="res")
            nc.scalar.activation(out=res[:], in_=u[:],
                                 func=mybir.ActivationFunctionType.Sigmoid,
                                 scale=-SQRT2, bias=1.0)
            nc.sync.dma_start(out=out[:], in_=res[:])
    nc.compile()
    return nc


def _get_nc():
    if "nc" not in _cache:
        _cache["nc"] = _build()
    return _cache["nc"]


def _host_inputs(zf, edge_index):
    zb = np.asarray(zf, dtype=np.float32).astype(ml_dtypes.bfloat16)
    src = np.asarray(edge_index[0]).astype(np.int64)
    dst = np.asarray(edge_index[1]).astype(np.int64)
    in_maps = []
    for c in range(N_CORES):
        lo, hi = c * EPC, (c + 1) * EPC
        s = np.zeros(T * P, dtype=np.int64)
        d = np.zeros(T * P, dtype=np.int64)
        s[:EPC] = src[lo:hi]
        d[:EPC] = dst[lo:hi]
        iav = s.reshape(T, P).T.astype(np.int32).copy()
        ibv = d.reshape(T, P).T.astype(np.int32).copy()
        in_maps.append({"z": zb, "ia": iav, "ib": ibv})
    return in_maps


def _run(z, edge_index, trace=False, tmpdir=None):
    nc = _get_nc()
    in_maps = _host_inputs(z, edge_index)
    res = run_bass_kernel_spmd(
        nc, in_maps, core_ids=list(range(N_CORES)), trace=trace, tmpdir=tmpdir)
    outs = []
    for c in range(N_CORES):
        o = res.results[c]["out"]          # [P, T]
        outs.append(np.asarray(o).T.reshape(-1)[:EPC])
    return np.concatenate(outs).astype(np.float32), res


def kernel(z, edge_index):
    out, _ = _run(z, edge_index)
    return out
